# revision 38
# baseline (speedup 1.0000x reference)
"""CASSI GAP reconstruction (DifferentiableGAPTV) on 8 Trainium2 NeuronCores.

Sharding: H=512 rows across 8 cores as 128-row slabs (64 output rows + 32-row
halo each side).  Rows couple only through the depthwise conv (+-1 row/iter
with the 3-tap approximation, 12 iters -> 12 rows), so the halo makes the
whole run collective-free; each core's central 64 rows are exact.

Numerics: the 5x5 Gaussian (sigma=0.5) is separable with 1-D taps
[2.6e-4, 0.107, 0.786, 0.107, 2.6e-4]; dropping the outer taps and
renormalizing (3-tap) plus keeping all per-band state in fp16 gives a
rel-err of ~3e-3 vs the exact reference (gate is 2e-2), while enabling:
  - fp16 matmuls (1 cycle/row at any free size, no f32r alignment rules)
  - DVE 2x perf mode on all elementwise tensor_tensor ops
  - halved SBUF footprint

Per-iteration structure (bands grouped by 4 for DVE, by 2 for PSUM):
  A:  v_l = m .* x_l          (DVE/Pool fp16 mults, batched over 4 bands)
      yb += shift_l(v_l)      (PE identity matmuls accumulating a PSUM plane;
                               band 0 scattered first w/ start=True, band 1
                               last so the final scatter's operand is ready
                               long before, shrinking the iteration-boundary
                               PE bubble)
  B:  t0 = z - 2*yb (fp16), zt = y - yb, z += zt   (z := y1 + y invariant)
  C:  w_l = x_l + mi_l .* t0[d_l:d_l+512]          (DVE fp16, batched)
      x2_l = sum_dc (g[dc]*B3)^T w_l(<<dc)         (3 PE matmuls/band, PSUM)
      x_l = fp16(x2_l)                             (ACT copy, batched per pair)
Final iteration DMAs x2 (f32) straight from PSUM to DRAM.
"""
import sys

sys.path.insert(0, "/opt/trn_rl_repo")
import numpy as np
import concourse.bass as bass
import concourse.mybir as mybir
import concourse.tile as tile
from concourse.bass_types import AP
from concourse.bass_utils import run_bass_kernel_spmd

H, W, L = 512, 512, 28
N_ITER = 12
SIGMA = 0.5
PI = 3.141592653589793
NCORES = 8
ROWS = 128          # slab rows per core
OUT_ROWS = 64       # exact output rows per core
HALO = 32           # (ROWS - OUT_ROWS) / 2
WP = 516            # padded conv-input pitch (2 zero cols each side)

f32 = mybir.dt.float32
f16 = mybir.dt.float16

GRP = 4             # bands per DVE group
NGRP = L // GRP     # 7
# A-mult groups computed on the Pool engine (rest on DVE). Early groups:
# their ACT copies land first, so Pool's slow ops finish within the C phase.
AM_POOL = (0, 1, 2)


def _offsets(s, phi_deg):
    phi = phi_deg * PI / 180.0
    dx = s * np.cos(phi)
    dy = s * np.sin(phi)
    dx = dx - dx.min()
    dy = dy - dy.min()
    return np.rint(dx).astype(np.int32), np.rint(dy).astype(np.int32)


def _gauss3(sigma):
    ksize = max(3, int(6 * sigma + 1) | 1)
    ax = np.arange(ksize, dtype=np.float32) - ksize // 2
    g1 = np.exp(-0.5 * (ax / sigma) ** 2)
    g1 = g1 / g1.sum()
    c = ksize // 2
    g3 = g1[c - 1 : c + 2].astype(np.float64)
    g3 = (g3 / g3.sum()).astype(np.float32)
    return g3  # [a1, a0, a1]


def _split_excess_waits(nc, max_w=1):
    """walrus accepts at most one sync wait per instruction; hoist excess
    waits onto preceding same-engine NoOp carriers."""
    ctr = 0
    for f in nc.m.functions:
        for bb in f.blocks:
            il = bb.instructions
            i = 0
            while i < len(il):
                inst = il[i]
                si = inst.sync_info
                w = list(si.on_wait) if (si and si.on_wait) else []
                if len(w) > max_w:
                    si.on_wait = w[-max_w:]
                    extra = w[:-max_w]
                    pos = i
                    for j in range(0, len(extra), max_w):
                        ctr += 1
                        nop = mybir.InstNoOp(
                            name=f"I-waitsplit-{ctr}", ins=[], outs=[]
                        )
                        nop.engine = inst.engine
                        nop.sync_info = mybir.SyncInfo(
                            on_wait=extra[j : j + max_w], on_update=[]
                        )
                        il.insert(pos, nop)
                        pos += 1
                        i += 1
                i += 1


def _gview(t, g, n=GRP, w=W):
    """Overlapping gather view: [128, n, w] over a [128, >=w+n*GRP] plane,
    band k of group g reading cols [g*n+k, g*n+k+w). Requires shift d_l == l."""
    base = t[:, 0:w]          # [ (pitch,128), (1,w) ]
    pitch = base.ap[0][0]
    return AP(tensor=base.tensor, offset=g * n,
              ap=[[pitch, 128], [1, n], [1, w]])


def build_nc(n_iter=N_ITER):
    nc = bass.Bass()
    y32_in = nc.declare_dram_parameter("y32", [ROWS, 539], f32, isOutput=False)
    y16_in = nc.declare_dram_parameter("y16", [ROWS, 539], f16, isOutput=False)
    m16_in = nc.declare_dram_parameter("m16", [ROWS, W], f16, isOutput=False)
    m2_in = nc.declare_dram_parameter("m2_16", [ROWS, W], f16, isOutput=False)
    # weights: [I, a0*B3, a1*B3] stacked -> [128, 3, 128] fp16
    w16_in = nc.declare_dram_parameter("w16", [128, 3, 128], f16, isOutput=False)
    mi_in = nc.declare_dram_parameter("mi16", [ROWS, L, W], f16, isOutput=False)
    out = nc.declare_dram_parameter("xout", [L, OUT_ROWS, W], f32, isOutput=True)

    Wm = 539

    with tile.TileContext(nc) as tc:
        with (
            tc.tile_pool(name="state", bufs=1) as st,
            tc.tile_pool(name="ybps", bufs=1, space="PSUM") as ybp,
            tc.tile_pool(name="cps", bufs=3, space="PSUM") as cp,
        ):
            # ---- persistent SBUF state ----
            y32 = st.tile([ROWS, Wm], f32)
            y16 = st.tile([ROWS, Wm], f16)
            m16 = st.tile([ROWS, W], f16)
            m2 = st.tile([ROWS, W], f16)
            w16 = st.tile([128, 3, 128], f16)
            mi = st.tile([ROWS, L, W], f16)
            xs = st.tile([ROWS, L, W], f16)
            vall = st.tile([ROWS, L, W], f16)
            t016 = st.tile([ROWS, Wm], f16)
            z = st.tile([ROWS, Wm], f32)
            zt = st.tile([ROWS, Wm], f32)
            wg = [st.tile([ROWS, GRP, WP], f16, name=f"wg{i}") for i in range(2)]
            stg = [st.tile([ROWS, 2, W], f32, name=f"stg{i}") for i in range(4)]
            ztail = st.tile([128, 27], f16)

            W_I = w16[:, 0, :]
            W_C = w16[:, 1, :]
            W_S = w16[:, 2, :]

            # ---- load inputs (mi streamed in 7-band chunks for pipelining;
            # y32 before mi so z is ready for iteration 0's t0) ----
            nc.sync.dma_start(y16[:], y16_in[:])
            nc.sync.dma_start(m2[:], m2_in[:])
            nc.sync.dma_start(m16[:], m16_in[:])
            nc.sync.dma_start(w16[:], w16_in[:])
            nc.sync.dma_start(y32[:], y32_in[:])
            for c in range(4):
                nc.sync.dma_start(mi[:, 7 * c : 7 * (c + 1), :],
                                  mi_in[:, 7 * c : 7 * (c + 1), :])

            # conv-input pads stay zero forever
            for t in wg:
                nc.vector.memset(t[:, :, 0:2], 0.0)
                nc.vector.memset(t[:, :, 514:516], 0.0)
            nc.vector.memset(ztail[:], 0.0)

            m16b = m16[:, None, :].to_broadcast((ROWS, GRP, W))
            m2b = m2[:, None, :].to_broadcast((ROWS, GRP, W))

            # ---- v0 = m^2 .* gather(y) directly (feeds iter-0 scatter ASAP);
            #      x0 = m .* gather(y);  z = 2*y ----
            for g in range(NGRP):
                nc.vector.tensor_mul(out=vall[:, g * GRP : (g + 1) * GRP, :],
                                     in0=m2b, in1=_gview(y16, g))
            for g in range(NGRP):
                eng = nc.gpsimd if g >= 4 else nc.vector
                eng.tensor_mul(out=xs[:, g * GRP : (g + 1) * GRP, :],
                               in0=m16b, in1=_gview(y16, g))
            nc.vector.tensor_scalar_mul(z[:], y32[:], 2.0)

            # PE warmup: dummy matmuls into the (about-to-be-overwritten) yb
            # buffer while the setup DMAs/DVE run.  They keep PE continuously
            # busy into iteration 0, so the pstate ramp reaches peak clock
            # before the real scatters and convs start.
            ybw = ybp.tile([ROWS, 544], f32, tag="yb")
            for _ in range(14):
                nc.tensor.matmul(ybw[:, 0:512], W_I, y16[:, 0:512],
                                 start=True, stop=True, skip_group_check=True)

            # scatter order: band 0 first (start=True covers cols [0,512)),
            # ascending after that (iter 0 chases the setup A-mult groups),
            # band 1 last so the final scatter operand is ready long before.
            scat_order = [0] + list(range(2, L)) + [1]

            for it in range(n_iter):
                # ---- A: yb = sum_l shift_l(v_l) ----
                yb = ybp.tile([ROWS, 544], f32, tag="yb")
                # zero-init the tail region [512, 539) so band tails can
                # accumulate in any order
                nc.tensor.matmul(yb[:, 512:539], W_I, ztail[:],
                                 start=True, stop=False, skip_group_check=True)
                for l in scat_order:
                    v = vall[:, l, :]
                    nc.tensor.matmul(
                        yb[:, l : 512], W_I, v[:, : W - l],
                        start=(l == 0), stop=(l == 1), skip_group_check=True,
                    )
                    if l > 0:
                        nc.tensor.matmul(
                            yb[:, 512 : 512 + l], W_I, v[:, W - l :],
                            start=False, stop=(l == 1),
                            skip_group_check=True,
                        )

                # ---- B+C, ordered to keep PE busy across the boundary ----
                last = it == n_iter - 1

                def conv_pair(w, j, b0):
                    """normal conv of the pair (w padded), returns x2 tile"""
                    x2 = cp.tile([ROWS, 2, W], f32, tag="x2")
                    for k in (0, 1):
                        wb = w[:, 2 * j + k, :]
                        nc.tensor.matmul(x2[:, k, :], W_S, wb[:, 1:513],
                                         start=True, stop=False,
                                         skip_group_check=True)
                        nc.tensor.matmul(x2[:, k, :], W_S, wb[:, 3:515],
                                         start=False, stop=False,
                                         skip_group_check=True)
                        nc.tensor.matmul(x2[:, k, :], W_C, wb[:, 2:514],
                                         start=False, stop=True,
                                         skip_group_check=True)
                    return x2

                def emit_out(x2, b0, j):
                    if last:
                        s = stg[(b0 // 2 + j) % 4]
                        nc.scalar.copy(s[:], x2[:])
                        for k in (0, 1):
                            l = b0 + 2 * j + k
                            nc.sync.dma_start(
                                out[l, :, :], s[HALO : HALO + OUT_ROWS, k, :]
                            )
                    else:
                        nc.scalar.copy(
                            xs[:, b0 + 2 * j : b0 + 2 * j + 2, :], x2[:]
                        )

                # t0 = z - 2*yb (fp16) -- the only op gating the C phase
                nc.vector.scalar_tensor_tensor(
                    out=t016[:], in0=yb[:, :Wm], scalar=-2.0, in1=z[:],
                    op0=mybir.AluOpType.mult, op1=mybir.AluOpType.add,
                )

                # pair 0 (bands 0,1): split conv(x + p) = conv(x) + conv(p).
                # The conv(x) taps have no dependence on t0, so PE chews on
                # them while DVE computes t0 and p01; no PE idle, no pstate
                # drop. p01 lives in wg[0][:, 0:2] (padded, no x add needed).
                w0 = wg[0]
                x2p0 = cp.tile([ROWS, 2, W], f32, tag="x2")
                for k in (0, 1):
                    xb = xs[:, k, :]
                    nc.tensor.matmul(x2p0[:, k, :], W_C, xb,
                                     start=True, stop=False,
                                     skip_group_check=True)
                    nc.tensor.matmul(x2p0[:, k, 0:511], W_S, xb[:, 1:512],
                                     start=False, stop=False,
                                     skip_group_check=True)
                    nc.tensor.matmul(x2p0[:, k, 1:512], W_S, xb[:, 0:511],
                                     start=False, stop=False,
                                     skip_group_check=True)
                nc.vector.tensor_mul(out=w0[:, 0:2, 2 : 2 + W],
                                     in0=mi[:, 0:2, :],
                                     in1=_gview(t016, 0, n=2))
                for k in (0, 1):
                    pb = w0[:, k, :]
                    nc.tensor.matmul(x2p0[:, k, :], W_S, pb[:, 1:513],
                                     start=False, stop=False,
                                     skip_group_check=True)
                    nc.tensor.matmul(x2p0[:, k, :], W_S, pb[:, 3:515],
                                     start=False, stop=False,
                                     skip_group_check=True)
                    nc.tensor.matmul(x2p0[:, k, :], W_C, pb[:, 2:514],
                                     start=False, stop=True,
                                     skip_group_check=True)
                emit_out(x2p0, 0, 0)

                # pair 1 (bands 2,3): normal path, 2-band group
                nc.vector.tensor_mul(out=w0[:, 2:4, 2 : 2 + W],
                                     in0=mi[:, 2:4, :],
                                     in1=AP(tensor=t016[:, 0:W].tensor,
                                            offset=2,
                                            ap=[[t016[:, 0:W].ap[0][0], 128],
                                                [1, 2], [1, W]]))
                nc.vector.tensor_add(out=w0[:, 2:4, 2 : 2 + W],
                                     in0=w0[:, 2:4, 2 : 2 + W],
                                     in1=xs[:, 2:4, :])
                x2 = conv_pair(w0, 1, 0)
                emit_out(x2, 0, 1)

                # next iteration's v for bands 0-3: on Pool, right after the
                # pair-0/1 copies land, so Pool's slow ops finish in-phase
                if not last:
                    nc.gpsimd.tensor_mul(out=vall[:, 0:GRP, :], in0=m16b,
                                         in1=xs[:, 0:GRP, :])

                # 4-band groups: bands 4..27
                for g in range(1, NGRP):
                    b0 = g * GRP
                    w = wg[g % 2]
                    nc.vector.tensor_mul(out=w[:, :, 2 : 2 + W],
                                         in0=mi[:, b0 : b0 + GRP, :],
                                         in1=_gview(t016, g))
                    nc.vector.tensor_add(out=w[:, :, 2 : 2 + W],
                                         in0=w[:, :, 2 : 2 + W],
                                         in1=xs[:, b0 : b0 + GRP, :])
                    if g == 3 and not last:
                        # z += y - yb; here the DVE supply runs well ahead
                        # of PE, so this doesn't stall the conv stream
                        nc.vector.scalar_tensor_tensor(
                            out=zt[:], in0=yb[:, :Wm], scalar=-1.0,
                            in1=y32[:],
                            op0=mybir.AluOpType.mult, op1=mybir.AluOpType.add,
                        )
                    if g == 6 and not last:
                        # AM3 squeezed in before group 6's w ops: its copies
                        # (bands 12-15) landed long ago, and emitting it here
                        # (instead of in the tail) lets the scatter of bands
                        # 12-15 start on time next iteration
                        nc.vector.tensor_mul(out=vall[:, 12:16, :],
                                             in0=m16b, in1=xs[:, 12:16, :])
                    for j in (0, 1):
                        x2 = conv_pair(w, j, b0)
                        emit_out(x2, b0, j)
                    if not last and g in AM_POOL:
                        # Pool A-mults trail their group's copies directly
                        nc.gpsimd.tensor_mul(out=vall[:, b0 : b0 + GRP, :],
                                             in0=m16b,
                                             in1=xs[:, b0 : b0 + GRP, :])
                if not last:
                    # DVE A-mults for the late groups, after the CM/CA stream
                    for ag in (4, 5, 6):
                        a0 = ag * GRP
                        nc.vector.tensor_mul(out=vall[:, a0 : a0 + GRP, :],
                                             in0=m16b,
                                             in1=xs[:, a0 : a0 + GRP, :])
                    # z-add last in the Pool stream (needed only by next t0)
                    nc.gpsimd.tensor_add(out=z[:], in0=z[:], in1=zt[:])

    _split_excess_waits(nc, max_w=1)
    return nc


def _host_inputs(y_1hw, mask2d, dx):
    y2 = np.asarray(y_1hw, dtype=np.float32)[0]      # [512, 539]
    m2 = np.asarray(mask2d, dtype=np.float32)        # [512, 512]
    Wm = W + int(max(dx))
    g3 = _gauss3(SIGMA)
    ident = np.eye(128, dtype=np.float16)

    # Phi_sum / inv-phi in f32 (matches reference construction)
    phi = np.zeros((H, Wm), dtype=np.float32)
    for l in range(L):
        phi[:, dx[l] : dx[l] + W] += m2
    invphi = 1.0 / np.maximum(phi, 1.0)

    in_maps = []
    for c in range(NCORES):
        rk = 64 * c - HALO
        y_slab = np.zeros((ROWS, Wm), dtype=np.float32)
        m_slab = np.zeros((ROWS, W), dtype=np.float32)
        mi_slab = np.zeros((ROWS, L, W), dtype=np.float16)
        lo = max(0, -rk)              # first valid slab row
        hi = min(ROWS, H - rk)        # one past last valid slab row
        y_slab[lo:hi] = y2[rk + lo : rk + hi]
        m_slab[lo:hi] = m2[rk + lo : rk + hi]
        for l in range(L):
            mi_slab[lo:hi, l, :] = (
                m_slab[lo:hi] * invphi[rk + lo : rk + hi, dx[l] : dx[l] + W]
            ).astype(np.float16)
        # banded 3-tap row-conv matrix, zeroed outside the valid row range
        B = np.zeros((128, 128), dtype=np.float32)
        for k in range(-1, 2):
            for i in range(128):
                ip = i + k
                if lo <= i < hi and lo <= ip < hi:
                    B[ip, i] = g3[k + 1]
        wm = np.zeros((128, 3, 128), dtype=np.float16)
        wm[:, 0, :] = ident
        wm[:, 1, :] = (g3[1] * B).astype(np.float16)
        wm[:, 2, :] = (g3[0] * B).astype(np.float16)
        m_16 = m_slab.astype(np.float16)
        in_maps.append({
            "y32": y_slab,
            "y16": y_slab.astype(np.float16),
            "m16": m_16,
            "m2_16": (m_16.astype(np.float32) ** 2).astype(np.float16),
            "w16": wm,
            "mi16": mi_slab,
        })
    return in_maps


_NC_CACHE = {}


def _get_nc(dx, n_iter=N_ITER):
    del dx
    key = n_iter
    if key not in _NC_CACHE:
        _NC_CACHE[key] = build_nc(n_iter)
    return _NC_CACHE[key]


def kernel(y_1hw, mask2d, phi_d_deg, s_nom, n_iter=N_ITER, trace=False):
    s = np.asarray(s_nom, dtype=np.float32)
    phi = float(np.asarray(phi_d_deg))
    dx, dy = _offsets(s, phi)
    assert (dy == 0).all(), "kernel assumes dy == 0"
    assert (dx == np.arange(L)).all(), "kernel assumes unit dispersion steps"
    nc = _get_nc(dx, n_iter)
    in_maps = _host_inputs(y_1hw, mask2d, dx)
    res = run_bass_kernel_spmd(nc, in_maps, list(range(NCORES)), trace=trace)
    x_full = np.empty((1, L, H, W), dtype=np.float32)
    for c in range(NCORES):
        x_full[0, :, 64 * c : 64 * (c + 1), :] = res.results[c]["xout"]
    kernel.last_results = res
    return x_full
